# revision 1
# baseline (speedup 1.0000x reference)
"""Trainium2 Bass kernel: causal GQA self-attention, RoPE + QK RMS-norm, bf16.

Sharding over 8 NeuronCores: core = 4*b + g (b in {0,1} batch, g in {0..3}
kv-group). Each core computes its 4 q heads + 1 kv head and the partial
c_proj output y_heads @ wproj[:, 512g:512g+512].T of shape [T, C]; the host
sums the 4 partials per batch (the "all-reduce after c_proj" at gather time).

Per-core program (bf16 stationary/moving matmuls, f32 PSUM accumulate):
  stage A per 128-row strip i: Q/KV projections (stationary = xT chunk shared
    between the interleaved Q and KV matmuls), RoPE + RMS-norm on DVE/ACT in
    f32, normalized q/k transposed to [d, t] via DMA-xbar transposes.
  stage B per (tq-slice j, head h): all S^T = K^T.Q matmuls + exps first
    (pe tiles persistent), then the denominator sweep (ones-column matmuls),
    then the AV sweep; denominator reciprocal runs on DVE overlapped with the
    AV sweep, and the 1/denom broadcast matmul + in-place normalize of each
    head is deferred by one head so PE never waits on the chain.
  stage C per j: c_proj, psum->sbuf copies alternating ACT/DVE, output DMA.

Startup DMA order matters (the SP queue is serial): first x strip + cos/sin
first, then mask/ident + wq/wkv chunks; wproj chunks stream during stage A.
"""

import math
from contextlib import ExitStack

import numpy as np

import concourse.bass as bass
import concourse.mybir as mybir
import concourse.tile as tile
from concourse import bacc
from concourse.bass import ts
from concourse.bass_utils import run_bass_kernel_spmd

F32 = mybir.dt.float32
BF16 = mybir.dt.bfloat16
N_HEAD = 16
N_KV = 4
D = 128
RMS_EPS = float(np.finfo(np.float32).eps)
SCALE = 1.0 / math.sqrt(D)


def build_bass(T=2048, C=2048, HQ=4, E=2048, rep=1, dt=BF16,
               pscfg=None, pbufs=16, obufs=3,
               gps=False, bpd=False, stages='ABC',
               xbar=True, absr=True, wsplit=4, gdma=True, ttr=False,
               defnorm=True):
    TT, CT, NE, TQ = T // 128, C // 128, E // 512, T // 512
    HD = HQ * 128
    if pscfg is None:
        pscfg = (5, 0, 2, 1) if xbar else (3, 2, 2, 1)

    nc = bacc.Bacc("TRN2", target_bir_lowering=False)
    xT_d = nc.dram_tensor("xT", [C, T], dt, kind="ExternalInput")
    wqT_d = nc.dram_tensor("wqT", [C, HD], dt, kind="ExternalInput")
    wkvT_d = nc.dram_tensor("wkvT", [C, 256], dt, kind="ExternalInput")
    wpT_d = nc.dram_tensor("wpT", [HD, E], dt, kind="ExternalInput")
    cos_d = nc.dram_tensor("cosd", [T, D], F32, kind="ExternalInput")
    sin_d = nc.dram_tensor("sind", [T, D], F32, kind="ExternalInput")
    mask_d = nc.dram_tensor("maskd", [128, 128], dt, kind="ExternalInput")
    id_d = nc.dram_tensor("identd", [128, 128], dt, kind="ExternalInput")
    out_d = nc.dram_tensor("out", [T, E], F32, kind="ExternalOutput")

    with tile.TileContext(nc) as tc, ExitStack() as ctx:
        P = lambda **kw: ctx.enter_context(tc.tile_pool(**kw))
        wp = P(name="w", bufs=1)            # persistent weights/constants
        xp = P(name="x", bufs=2)            # xT strips
        csp = P(name="cs", bufs=2)          # cos/sin tiles
        rp = P(name="rope", bufs=2)         # rope scratch
        qnp = P(name="qn", bufs=2)          # normalized q/k (pre-transpose)
        pp = P(name="p", bufs=pbufs)        # exp(P) tiles (all blocks alive)
        bp = P(name="bc", bufs=2)           # denominators / bcast
        yp = P(name="y", bufs=2)            # per-j YT
        op = P(name="o", bufs=obufs)        # output staging
        sb_, tb_, ab_, db_ = pscfg
        ps_s = P(name="ps_s", bufs=sb_, space="PSUM")   # proj / scores / cproj
        ps_t = None
        if tb_:
            ps_t = P(name="ps_t", bufs=tb_, space="PSUM")   # transposes
        ps_a = P(name="ps_a", bufs=ab_, space="PSUM")   # AV accumulators
        ps_d = P(name="ps_d", bufs=db_, space="PSUM")   # denominators

        # persistent SBUF; DMA order matters: the SP queue is serial, so the
        # first x strip + cos/sin go first, then mask/ident and the weight
        # chunks; wproj is streamed chunk-wise during early stage A (first
        # read in stage C)
        xT_r0 = xT_d.ap().rearrange("(n p) t -> p n t", p=128)
        xs0 = xp.tile([128, CT, 128], dt, name="xs0", tag="xs0", bufs=1)
        nc.sync.dma_start(xs0, xT_r0[:, :, ts(0, 128)])
        cst0 = csp.tile([128, D], F32, name="cst0", tag="cst0", bufs=1)
        nc.sync.dma_start(cst0, cos_d.ap()[ts(0, 128), :])
        snt0 = csp.tile([128, D], F32, name="snt0", tag="snt0", bufs=1)
        nc.sync.dma_start(snt0, sin_d.ap()[ts(0, 128), :])
        mask_s = wp.tile([128, 128], dt)
        nc.sync.dma_start(mask_s, mask_d.ap())
        ident = wp.tile([128, 128], dt)
        nc.sync.dma_start(ident, id_d.ap())
        wq_s = wp.tile([128, CT, HD], dt)
        wkv_s = wp.tile([128, CT, 256], dt)
        wq_r = wqT_d.ap().rearrange("(n p) m -> p n m", p=128)
        wkv_r = wkvT_d.ap().rearrange("(n p) m -> p n m", p=128)
        csz = CT // wsplit
        for w in range(wsplit):
            cs0 = w * csz
            nc.sync.dma_start(wq_s[:, cs0:cs0 + csz], wq_r[:, cs0:cs0 + csz])
            nc.sync.dma_start(wkv_s[:, cs0:cs0 + csz], wkv_r[:, cs0:cs0 + csz])
        wp_s = wp.tile([128, HQ, E], dt)
        wp_r = wpT_d.ap().rearrange("(n p) m -> p n m", p=128)
        wp_loaded = [False]
        ones_c = wp.tile([128, 1], dt, name="ones_c", tag="ones_c")
        nc.vector.memset(ones_c, 1.0)
        ones_sq = wp.tile([128, 128], dt, name="ones_sq", tag="ones_sq")
        nc.vector.memset(ones_sq, 1.0)
        eps_s = wp.tile([128, 1], F32)
        nc.vector.memset(eps_s, RMS_EPS)

        def bcast(ap, axis, n):
            a = list(ap.ap)
            a.insert(axis, [0, n])
            return bass.AP(tensor=ap.tensor, offset=ap.offset, ap=a)

        for _ in range(rep):
            qT = {}  # (h, j) -> [128, 4, 128] tile, d-major
            kT = []  # i -> [128, 128]
            vS = []  # i -> [128, 128]
            for h in range(HQ):
                for j in range(TQ):
                    qT[(h, j)] = wp.tile([128, 4, 128], dt, tag=f"qT{h}_{j}",
                                         name=f"qT{h}_{j}")
            for i in range(TT):
                kT.append(wp.tile([128, 128], dt, tag=f"kT{i}", name=f"kT{i}"))
                vS.append(wp.tile([128, 128], dt, tag=f"vS{i}", name=f"vS{i}"))

            # ---- stage A: projections + rope + rms + transpose ----
            xT_r = xT_d.ap().rearrange("(n p) t -> p n t", p=128)
            pend = []  # deferred transposes: (qn_ap, dst)

            def drain_transposes():
                for src_ap, dst in pend:
                    if xbar:
                        nc.sync.dma_start(dst, src_ap, transpose=True)
                    else:
                        pt = ps_t.tile([128, 128], dt)
                        nc.tensor.transpose(pt, src_ap, ident)
                        nc.vector.tensor_copy(dst, pt)
                del pend[:]

            def rope_rms(src, nh, dst_list, cst, snt):
                """src: psum AP viewed [128, nh, 128]; queue transposes."""
                ro = rp.tile([128, nh, D], F32, tag=f"ro{nh}", name="ro")
                nc.vector.tensor_mul(ro, src, bcast(cst[:, :], 1, nh))
                tmp = rp.tile([128, nh, 64], F32, tag=f"tm{nh}", name="tmp")
                nc.vector.tensor_mul(tmp, src[:, :, 64:128],
                                     bcast(snt[:, 0:64], 1, nh))
                nc.vector.tensor_sub(ro[:, :, 0:64], ro[:, :, 0:64], tmp)
                tmp2 = rp.tile([128, nh, 64], F32, tag=f"t2{nh}", name="tmp2")
                nc.vector.tensor_mul(tmp2, src[:, :, 0:64],
                                     bcast(snt[:, 64:128], 1, nh))
                nc.vector.tensor_add(ro[:, :, 64:128], ro[:, :, 64:128],
                                     tmp2)
                scr = rp.tile([128, nh, D], F32, tag=f"sc{nh}", name="scr")
                sq = rp.tile([128, nh], F32, tag=f"sq{nh}", name="sq")
                if ttr:
                    for h in range(nh):
                        nc.vector.tensor_tensor_reduce(
                            scr[:, h], ro[:, h], ro[:, h], 1.0, 0.0,
                            mybir.AluOpType.mult, mybir.AluOpType.add,
                            sq[:, h:h + 1])
                else:
                    nc.vector.tensor_mul(scr, ro, ro)
                    nc.vector.reduce_sum(sq, scr, axis=mybir.AxisListType.X)
                rr = rp.tile([128, nh], F32, tag=f"rr{nh}", name="rr")
                if absr:
                    nc.scalar.activation(
                        rr, sq,
                        mybir.ActivationFunctionType.Abs_reciprocal_sqrt,
                        bias=eps_s[:, :], scale=1.0 / D)
                else:
                    rs = rp.tile([128, nh], F32, tag=f"rs{nh}", name="rs")
                    nc.scalar.activation(rs, sq,
                                         mybir.ActivationFunctionType.Sqrt,
                                         bias=eps_s[:, :], scale=1.0 / D)
                    nc.vector.reciprocal(rr, rs)
                qn = qnp.tile([128, nh, D], dt, tag=f"qn{nh}", name="qn")
                qeng = nc.gpsimd if gps else nc.vector
                for h in range(nh):
                    qeng.tensor_scalar_mul(qn[:, h], ro[:, h],
                                           rr[:, h:h + 1])
                    pend.append((qn[:, h], dst_list[h]))

            def stage_a(i):
                if i == 0 and not wp_loaded[0]:
                    xs, cst, snt = xs0, cst0, snt0
                else:
                    xs = xp.tile([128, CT, 128], dt, name="xs", tag="xs")
                    nc.sync.dma_start(xs, xT_r[:, :, ts(i, 128)])
                    cst = csp.tile([128, D], F32, tag="cos", name="cst")
                    (nc.gpsimd if gdma else nc.sync).dma_start(
                        cst, cos_d.ap()[ts(i, 128), :])
                    snt = csp.tile([128, D], F32, tag="sin", name="snt")
                    (nc.gpsimd if gdma else nc.sync).dma_start(
                        snt, sin_d.ap()[ts(i, 128), :])
                if not wp_loaded[0] and i >= 1:
                    lastw = i == TT - 1
                    hi = HQ if (i >= HQ or lastw) else i
                    for w in range(i - 1, hi):
                        nc.sync.dma_start(wp_s[:, w], wp_r[:, w])
                    if i >= HQ or lastw:
                        wp_loaded[0] = True

                pq = ps_s.tile([128, HD], F32, tag="s", name="pq")
                pkv = ps_s.tile([128, 256], F32, tag="s", name="pkv")
                for c in range(CT):
                    nc.tensor.matmul(pq, xs[:, c], wq_s[:, c],
                                     start=(c == 0), stop=(c == CT - 1))
                    nc.tensor.matmul(pkv, xs[:, c], wkv_s[:, c],
                                     start=(c == 0), stop=(c == CT - 1))
                drain_transposes()
                nc.scalar.copy(vS[i], pkv[:, 128:256])
                j, tsub = i // 4, i % 4
                rope_rms(pq[:].rearrange("p (h d) -> p h d", d=D), HQ,
                         [qT[(h, j)][:, tsub] for h in range(HQ)], cst, snt)
                rope_rms(pkv[:, 0:128].rearrange("p (h d) -> p h d", d=D), 1,
                         [kT[i]], cst, snt)

            # ---- stage B + C per tq-slice ----
            def stage_bc(j):
                nblk = 4 * j + 4
                ynj = yp.tile([128, HQ, 4, 128], dt)
                pd = None
                if bpd:
                    pd = ps_d.tile([128, 512], F32, name="pd", tag="pd")
                pend_norm = []  # (rdr, yv) deferred one head for slack
                for h in range(HQ):
                    pes = []
                    for i in range(nblk):
                        ai = max(0, i - 4 * j) * 128
                        psb = ps_s.tile([128, 512], F32, tag="s")
                        nc.tensor.matmul(psb[:, ai:512], kT[i],
                                         qT[(h, j)][:, ai // 128:4])
                        pe = pp.tile([128, 512], dt, tag="pe")
                        nc.scalar.activation(pe[:, ai:512], psb[:, ai:512],
                                             mybir.ActivationFunctionType.Exp,
                                             scale=SCALE)
                        if i >= 4 * j:
                            meng = nc.gpsimd if gps else nc.vector
                            meng.tensor_mul(pe[:, ai:ai + 128],
                                            pe[:, ai:ai + 128], mask_s)
                        pes.append((pe, ai))
                    if bpd:
                        for i, (pe, ai) in enumerate(pes):
                            nc.tensor.matmul(
                                pd[32 * h:32 * h + 1, ai:512],
                                ones_c, pe[:, ai:512],
                                start=(h == 0 and i == 0),
                                stop=(h == HQ - 1 and i == nblk - 1),
                                skip_group_check=True,
                                tile_position=(0, 32 * h))
                    else:
                        pdh = ps_d.tile([1, 512], F32)
                        for i, (pe, ai) in enumerate(pes):
                            nc.tensor.matmul(pdh[:, ai:512],
                                             ones_c, pe[:, ai:512],
                                             start=(i == 0),
                                             stop=(i == nblk - 1))
                    if not bpd:
                        # denominator chain on DVE only (on ACT it queues
                        # behind this head's exps); pb+normalize deferred to
                        # after the NEXT head's d-sweep so rdr has a full
                        # head of slack before PE consumes it
                        rd = bp.tile([1, 512], F32, tag="rd")
                        nc.vector.reciprocal(rd, pdh)
                        rdr = bp.tile([1, 512], dt, tag="rdr")
                        nc.vector.tensor_copy(rdr, rd)
                        if defnorm:
                            for rdr_p, yv_p in pend_norm:
                                pb = ps_s.tile([128, 512], F32, tag="s",
                                               name="pb")
                                nc.tensor.matmul(pb, ones_sq[0:1, :], rdr_p)
                                nc.vector.tensor_mul(yv_p, yv_p, pb)
                            del pend_norm[:]
                    pav = ps_a.tile([128, 512], F32)
                    for i, (pe, ai) in enumerate(pes):
                        nc.tensor.matmul(pav[:, ai:512], vS[i], pe[:, ai:512],
                                         start=(i == 0), stop=(i == nblk - 1))
                    # stash unnormalized AV in ynj (frees the psum bank
                    # without waiting on the denominator chain), then scale
                    # in place with 1/denom broadcast read from PSUM
                    yv = ynj[:, h].rearrange("p a b -> p (a b)")
                    if h % 2 == 0:
                        nc.scalar.copy(yv, pav)
                    else:
                        nc.vector.tensor_copy(yv, pav)
                    if not bpd:
                        if defnorm:
                            pend_norm.append((rdr, yv))
                        else:
                            pb = ps_s.tile([128, 512], F32, tag="s", name="pb")
                            nc.tensor.matmul(pb, ones_sq[0:1, :], rdr)
                            nc.vector.tensor_mul(yv, yv, pb)
                for rdr_p, yv_p in pend_norm:
                    pb = ps_s.tile([128, 512], F32, tag="s", name="pb")
                    nc.tensor.matmul(pb, ones_sq[0:1, :], rdr_p)
                    nc.vector.tensor_mul(yv_p, yv_p, pb)
                del pend_norm[:]
                if bpd:
                    rd = bp.tile([128, 512], F32, tag="rd")
                    nc.vector.reciprocal(rd, pd)
                    rdr = bp.tile([128, 512], dt, tag="rdr")
                    nc.scalar.copy(rdr, rd)
                    for hh in range(HQ):
                        pb = ps_s.tile([128, 512], F32, tag="s", name="pb")
                        nc.tensor.matmul(pb, ones_sq[32 * hh:32 * hh + 1, :],
                                         rdr[32 * hh:32 * hh + 1, :],
                                         tile_position=(32 * hh, 0))
                        yv = ynj[:, hh].rearrange("p a b -> p (a b)")
                        nc.vector.tensor_mul(yv, yv, pb)
                if stages == 'AB':
                    dbg2 = op.tile([128, 512], F32, tag="ot")
                    nc.vector.tensor_copy(
                        dbg2, ynj[:, 0].rearrange("p a b -> p (a b)"))
                    nc.sync.dma_start(out_d.ap()[ts(j, 128), 0:512], dbg2)
                    return
                for tsub in range(4):
                    for e in range(NE):
                        pc = ps_s.tile([128, 512], F32, tag="s", name="pc")
                        for h in range(HQ):
                            nc.tensor.matmul(pc, ynj[:, h, tsub],
                                             wp_s[:, h, ts(e, 512)],
                                             start=(h == 0), stop=(h == HQ - 1))
                        ot = op.tile([128, 512], F32, tag="ot", name="ot")
                        if e % 2 == 0:
                            nc.scalar.copy(ot, pc)
                        else:
                            nc.vector.tensor_copy(ot, pc)
                        deng = nc.sync if (e % 2 == 0 or not gdma) else nc.gpsimd
                        deng.dma_start(
                            out_d.ap()[512 * j + 128 * tsub:
                                       512 * j + 128 * tsub + 128,
                                       ts(e, 512)], ot)

            for i in range(TT):
                stage_a(i)
            drain_transposes()
            if stages == 'A':
                dbg = op.tile([128, 512], F32, tag="ot", name="dbg")
                nc.vector.tensor_copy(dbg[:, 0:128], kT[0])
                nc.sync.dma_start(out_d.ap()[0:128, 0:512], dbg)
                continue
            for j in range(TQ):
                stage_bc(j)
    nc.compile()
    return nc


def make_core_inputs(x, cos, sin, wq, wk, wv, wproj):
    """Full inputs -> list of 8 per-core input dicts (host-side sharding)."""
    bf16 = mybir.dt.np(BF16)
    x = np.asarray(x, dtype=np.float32)
    cos2 = np.ascontiguousarray(np.asarray(cos, np.float32).reshape(-1, D))
    sin2 = np.ascontiguousarray(np.asarray(sin, np.float32).reshape(-1, D))
    wq = np.asarray(wq, np.float32)
    wk = np.asarray(wk, np.float32)
    wv = np.asarray(wv, np.float32)
    wproj = np.asarray(wproj, np.float32)
    B = x.shape[0]
    mask = np.triu(np.ones((128, 128), np.float32)).astype(bf16)
    ident = np.eye(128, dtype=np.float32).astype(bf16)
    in_maps = []
    xTs = [np.ascontiguousarray(x[b].T).astype(bf16) for b in range(B)]
    for b in range(B):
        for g in range(N_KV):
            wqT = np.ascontiguousarray(wq[512 * g:512 * g + 512].T).astype(bf16)
            wkvT = np.ascontiguousarray(
                np.concatenate([wk[128 * g:128 * g + 128],
                                wv[128 * g:128 * g + 128]], axis=0).T).astype(bf16)
            wpT = np.ascontiguousarray(
                wproj[:, 512 * g:512 * g + 512].T).astype(bf16)
            in_maps.append({
                "xT": xTs[b], "wqT": wqT, "wkvT": wkvT, "wpT": wpT,
                "cosd": cos2, "sind": sin2, "maskd": mask, "identd": ident,
            })
    return in_maps


_NC_CACHE = {}


def kernel(x, cos, sin, wq, wk, wv, wproj):
    x = np.asarray(x, dtype=np.float32)
    B, T, C = x.shape
    key = (T, C)
    if key not in _NC_CACHE:
        _NC_CACHE[key] = build_bass(T=T, C=C)
    nc = _NC_CACHE[key]
    in_maps = make_core_inputs(x, cos, sin, wq, wk, wv, wproj)
    res = run_bass_kernel_spmd(nc, in_maps, core_ids=list(range(8)))
    out = np.zeros((B, T, C), dtype=np.float64)
    for b in range(B):
        for g in range(N_KV):
            out[b] += res.results[4 * b + g]["out"].astype(np.float64)
    return out.astype(np.float32)



# revision 24
# speedup vs baseline: 1.3312x; 1.3312x over previous
"""Trainium2 Bass kernel: causal GQA self-attention, RoPE + QK RMS-norm, bf16.

Sharding over 8 NeuronCores: core = 4*b + g (b in {0,1} batch, g in {0..3}
kv-group). Each core computes its 4 q heads + 1 kv head and the partial
c_proj output y_heads @ wproj[:, 512g:512g+512].T of shape [T, C]; the host
sums the 4 partials per batch (the "all-reduce after c_proj" at gather time).

Schedule (deep software pipeline): stage A of strip 4j+4+h is emitted inside
stage B(j) between heads h and h+1, so the serial per-strip chain
(PE proj -> DVE rope+rsqrt -> xbar transpose) hides under the PE-heavy
attention sweeps and every engine queue keeps flowing. Choices that matter:
 - rsqrt for the QK RMS-norm runs ENTIRELY on DVE (bit-trick seed + 2
   Newton steps on [128,5] tiles): the ACT engine then only ever runs
   Exp/Copy, which share one activation-table set -> zero ACT_TABLE_LOADs
   after the first.
 - causal mask is an additive -30000 matmul folded into the scores psum
   accumulation group on PE (no post-exp masking work on DVE/ACT).
 - q transposes are batched: one 3D-dst xbar DMA per strip covers all 4
   heads ([t,(h d)] -> [d,h,t]).
 - output stores are 4 batched [128,2048] DMAs per slice on the gpsimd
   (SWDGE) queue; cos/sin loads ride the same queue ahead of them; the SP
   (HWDGE) queue carries only x strips + transposes. In-order DMA queues
   head-of-line block on data dependencies, so queue assignment is part of
   the schedule.
 - fp8 (DoubleRow) paths exist but are OFF: every naive e4m3 insertion
   measures ~2.8e-2 rel err alone (gate 2e-2); error-feedback splits cost
   back the 2x.
"""

import math
from contextlib import ExitStack

import numpy as np

import concourse.bass as bass
import concourse.mybir as mybir
import concourse.tile as tile
from concourse import bacc
from concourse.bass import ts
from concourse.bass_utils import run_bass_kernel_spmd

F32 = mybir.dt.float32
I32 = mybir.dt.int32
BF16 = mybir.dt.bfloat16
FP8 = mybir.dt.float8e4
N_HEAD = 16
N_KV = 4
D = 128
RMS_EPS = float(np.finfo(np.float32).eps)
SCALE = 1.0 / math.sqrt(D)

FP8P = False         # x/wq/wkv in fp8, DoubleRow projections (see docstring)
FP8C = False         # wproj/ynj in fp8, DoubleRow c_proj
W_SCALE = 64.0
OUT_DT = "f32"
PEMASK = True        # mask = additive -30000 folded into scores matmul group

ALU = mybir.AluOpType
AF = mybir.ActivationFunctionType


def build_bass(T=2048, C=2048, HQ=4, E=2048, rep=1,
               pscfg=None, pbufs=16, obufs=3,
               stages='ABC', wsplit=4, gdma=True,
               defnorm=True, fp8p=None, fp8c=None, outdt=None,
               nrsqrt=True, deep=True, bigot=True, bxp=True,
               pemask=True, csq='gpsimd', stq='gpsimd', nriter=2):
    TT, CT, NE, TQ = T // 128, C // 128, E // 512, T // 512
    HD = HQ * 128
    if fp8p is None:
        fp8p = FP8P
    if fp8c is None:
        fp8c = FP8C
    if outdt is None:
        outdt = OUT_DT
    dt = BF16
    xdt = FP8 if fp8p else BF16
    ydt = FP8 if fp8c else BF16
    odt = F32 if outdt == "f32" else BF16
    os_val = (1.0 / W_SCALE) if fp8p else 1.0
    ot_scale = (1.0 / W_SCALE) if fp8c else None
    if pscfg is None:
        pscfg = (2, 2, 1, 2, 1)  # ps_s, pq, pkv, ps_a, ps_d (8 banks)
    sb_, qb_, kb_, ab_, db_ = pscfg

    nc = bacc.Bacc("TRN2", target_bir_lowering=False)
    xT_d = nc.dram_tensor("xT", [C, T], xdt, kind="ExternalInput")
    wqT_d = nc.dram_tensor("wqT", [C, HD], xdt, kind="ExternalInput")
    wkvT_d = nc.dram_tensor("wkvT", [C, 256], xdt, kind="ExternalInput")
    wpT_d = nc.dram_tensor("wpT", [HD, E], ydt, kind="ExternalInput")
    cos_d = nc.dram_tensor("cosd", [T, D], F32, kind="ExternalInput")
    sin_d = nc.dram_tensor("sind", [T, D], F32, kind="ExternalInput")
    mask_d = nc.dram_tensor("maskd", [128, 128], dt, kind="ExternalInput")
    id_d = nc.dram_tensor("identd", [128, 128], dt, kind="ExternalInput")
    out_d = nc.dram_tensor("out", [T, E], odt, kind="ExternalOutput")

    DR = mybir.MatmulPerfMode.DoubleRow
    assert pemask == PEMASK, "host mask content must match pemask"

    with tile.TileContext(nc) as tc, ExitStack() as ctx:
        P = lambda **kw: ctx.enter_context(tc.tile_pool(**kw))
        wp = P(name="w", bufs=1)            # persistent weights/constants
        xp = P(name="x", bufs=8)            # xT strips
        csp = P(name="cs", bufs=8)          # cos/sin tiles
        rp = P(name="rope", bufs=3)         # rope scratch
        qnp = P(name="qn", bufs=3)          # normalized q/k (pre-transpose)
        pp = P(name="p", bufs=pbufs)        # exp(P) tiles (all blocks alive)
        bp = P(name="bc", bufs=2)           # denominators / bcast
        yp = P(name="y", bufs=3)            # per-head unnormalized y^T
        op = P(name="o", bufs=obufs)        # output staging
        ps_s = P(name="ps_s", bufs=sb_, space="PSUM")   # scores/pb/cproj
        ps_q = P(name="ps_q", bufs=qb_, space="PSUM")   # q projection
        ps_k = P(name="ps_k", bufs=kb_, space="PSUM")   # kv projection
        ps_a = P(name="ps_a", bufs=ab_, space="PSUM")   # AV accumulators
        ps_d = P(name="ps_d", bufs=db_, space="PSUM")   # denominators

        engq = {"gpsimd": nc.gpsimd, "sp": nc.sync,
                "act": nc.scalar, "dve": nc.vector}
        cse = engq[csq] if gdma else nc.sync
        ste = engq[stq] if gdma else nc.sync

        xT_r = xT_d.ap().rearrange("(n p) t -> p n t", p=128)
        xs_t, cs_t = {}, {}

        def load_strip(i):
            xs = xp.tile([128, CT, 128], xdt, name="xs", tag="xs")
            nc.sync.dma_start(xs, xT_r[:, :, ts(i, 128)])
            cst = csp.tile([128, D], F32, tag="cos", name="cst")
            cse.dma_start(cst, cos_d.ap()[ts(i, 128), :])
            snt = csp.tile([128, D], F32, tag="sin", name="snt")
            cse.dma_start(snt, sin_d.ap()[ts(i, 128), :])
            xs_t[i] = xs
            cs_t[i] = (cst, snt)

        # ---- startup DMAs: first strip, then constants + weights ----
        load_strip(0)
        mask_s = wp.tile([128, 128], dt)
        nc.sync.dma_start(mask_s, mask_d.ap())
        ident = wp.tile([128, 128], dt)
        nc.sync.dma_start(ident, id_d.ap())
        wq_s = wp.tile([128, CT, HD], xdt)
        wkv_s = wp.tile([128, CT, 256], xdt)
        wq_r = wqT_d.ap().rearrange("(n p) m -> p n m", p=128)
        wkv_r = wkvT_d.ap().rearrange("(n p) m -> p n m", p=128)
        csz = CT // wsplit
        for w in range(wsplit):
            cs0 = w * csz
            nc.sync.dma_start(wq_s[:, cs0:cs0 + csz], wq_r[:, cs0:cs0 + csz])
            nc.sync.dma_start(wkv_s[:, cs0:cs0 + csz], wkv_r[:, cs0:cs0 + csz])
        for i in range(1, min(4, TT)):
            load_strip(i)
        wp_s = wp.tile([128, HQ, E], ydt)
        wp_r = wpT_d.ap().rearrange("(n p) m -> p n m", p=128)
        wp_loaded = [False]
        ones_c = wp.tile([128, 1], dt, name="ones_c", tag="ones_c")
        nc.vector.memset(ones_c, 1.0)
        ones_sq = wp.tile([128, 128], dt, name="ones_sq", tag="ones_sq")
        nc.vector.memset(ones_sq, os_val)
        eps_s = wp.tile([128, 1], F32)
        nc.vector.memset(eps_s, RMS_EPS)

        def bcast(ap, axis, n):
            a = list(ap.ap)
            a.insert(axis, [0, n])
            return bass.AP(tensor=ap.tensor, offset=ap.offset, ap=a)

        for r in range(rep):
            if r > 0:
                for i in range(min(4, TT)):
                    load_strip(i)
            if bxp:
                qT = {j: wp.tile([128, HQ, 4, 128], dt, tag=f"qTj{j}",
                                 name=f"qTj{j}") for j in range(TQ)}
            else:
                qT = {}
                for h in range(HQ):
                    for j in range(TQ):
                        qT[(h, j)] = wp.tile([128, 4, 128], dt,
                                             tag=f"qT{h}_{j}",
                                             name=f"qT{h}_{j}")
            kT = [wp.tile([128, 128], dt, tag=f"kT{i}", name=f"kT{i}")
                  for i in range(TT)]
            vS = [wp.tile([128, 128], dt, tag=f"vS{i}", name=f"vS{i}")
                  for i in range(TT)]

            pend = []  # deferred transposes: (src_ap, dst)

            def drain_transposes():
                for src_ap, dst in pend:
                    nc.sync.dma_start(dst, src_ap, transpose=True)
                del pend[:]

            def rope(src, nh, cst, snt, qn, qo):
                """src: psum [128, nh, 128] -> rope'd copy in qn[:, qo:qo+nh]
                (f32 scratch), returns sumsq written later; here returns ro."""
                ro = rp.tile([128, nh, D], F32, tag=f"ro{qo}", name="ro")
                nc.vector.tensor_mul(ro, src, bcast(cst[:, :], 1, nh))
                tmp = rp.tile([128, nh, 64], F32, tag=f"tm{qo}", name="tmp")
                nc.vector.tensor_mul(tmp, src[:, :, 64:128],
                                     bcast(snt[:, 0:64], 1, nh))
                nc.vector.tensor_sub(ro[:, :, 0:64], ro[:, :, 0:64], tmp)
                tmp2 = rp.tile([128, nh, 64], F32, tag=f"t2{qo}", name="tmp2")
                nc.vector.tensor_mul(tmp2, src[:, :, 0:64],
                                     bcast(snt[:, 64:128], 1, nh))
                nc.vector.tensor_add(ro[:, :, 64:128], ro[:, :, 64:128],
                                     tmp2)
                return ro

            MAGIC = 0x5F3759DF

            def nr_rsqrt(rr, v):
                """rr = 1/sqrt(v) elementwise on DVE only ([128, n] tiles).

                Bit-trick seed y0 via (M2 + ~i) >> 1 (= magic - (i>>1) up to
                1 ulp of the seed), then `nriter` Newton steps; no ACT
                involvement so the activation table stays on the Exp set.
                """
                n = v.shape[1]
                y = rp.tile([128, n], F32, tag="nr_y", name="nr_y")
                vb = v.bitcast(I32)
                yb = y.bitcast(I32)
                # walrus requires op0/op1 of one tensor_scalar to share a
                # class (bitwise vs arith), and there is no reversed
                # subtract: use magic - (i>>1) = ~(i>>1) + (magic+1)
                nc.vector.tensor_scalar(yb, vb, 1, 0xFFFFFFFF,
                                        ALU.logical_shift_right,
                                        ALU.bitwise_xor)
                nc.vector.tensor_scalar(yb, yb, MAGIC + 1, None, ALU.add)
                t = rp.tile([128, n], F32, tag="nr_t", name="nr_t")
                for _ in range(nriter):
                    nc.vector.tensor_mul(t, y, y)
                    nc.vector.tensor_mul(t, t, v)
                    nc.vector.tensor_scalar(t, t, -0.5, 1.5,
                                            ALU.mult, ALU.add)
                    nc.vector.tensor_mul(y, y, t)
                nc.vector.tensor_copy(rr, y)

            def stage_a(i):
                drain_transposes()
                xs = xs_t[i]
                cst, snt = cs_t[i]
                if not wp_loaded[0] and i >= 1:
                    # all HQ wproj chunks must be queued on the SP FIFO
                    # before bc(0)'s output stores (stage C(0) reads them)
                    hi = HQ if i >= 3 else i
                    for w in range(i - 1, hi):
                        nc.sync.dma_start(wp_s[:, w], wp_r[:, w])
                    if i >= 3:
                        wp_loaded[0] = True

                pq = ps_q.tile([128, HD], F32, tag="pq", name="pq")
                pkv = ps_k.tile([128, 256], F32, tag="pkv", name="pkv")
                if fp8p:
                    n2 = CT // 2
                    for c in range(n2):
                        sl = slice(2 * c, 2 * c + 2)
                        nc.tensor.matmul(pq, xs[:, sl], wq_s[:, sl],
                                         start=(c == 0), stop=(c == n2 - 1),
                                         perf_mode=DR)
                        nc.tensor.matmul(pkv, xs[:, sl], wkv_s[:, sl],
                                         start=(c == 0), stop=(c == n2 - 1),
                                         perf_mode=DR)
                else:
                    for c in range(CT):
                        nc.tensor.matmul(pq, xs[:, c], wq_s[:, c],
                                         start=(c == 0), stop=(c == CT - 1))
                        nc.tensor.matmul(pkv, xs[:, c], wkv_s[:, c],
                                         start=(c == 0), stop=(c == CT - 1))
                nc.scalar.copy(vS[i], pkv[:, 128:256])
                j, tsub = i // 4, i % 4

                ro_q = rope(pq[:].rearrange("p (h d) -> p h d", d=D), HQ,
                            cst, snt, None, 0)
                ro_k = rope(pkv[:, 0:128].rearrange("p (h d) -> p h d", d=D),
                            1, cst, snt, None, 8)
                sq5 = rp.tile([128, HQ + 1], F32, tag="sq5", name="sq5")
                if nrsqrt:
                    # v = mean(ro^2) + eps, rsqrt on DVE
                    scr = rp.tile([128, HQ, D], F32, tag="scr", name="scr")
                    nc.vector.tensor_mul(scr, ro_q, ro_q)
                    nc.vector.reduce_sum(sq5[:, 0:HQ], scr,
                                         axis=mybir.AxisListType.X)
                    scrk = rp.tile([128, 1, D], F32, tag="scrk", name="scrk")
                    nc.vector.tensor_mul(scrk, ro_k, ro_k)
                    nc.vector.reduce_sum(sq5[:, HQ:HQ + 1], scrk,
                                         axis=mybir.AxisListType.X)
                    nc.vector.tensor_scalar(sq5, sq5, 1.0 / D, RMS_EPS,
                                            ALU.mult, ALU.add)
                    rr5 = rp.tile([128, HQ + 1], F32, tag="rr5", name="rr5")
                    nr_rsqrt(rr5, sq5)
                else:
                    scr = rp.tile([128, HQ, D], F32, tag="scr", name="scr")
                    nc.vector.tensor_mul(scr, ro_q, ro_q)
                    nc.vector.reduce_sum(sq5[:, 0:HQ], scr,
                                         axis=mybir.AxisListType.X)
                    scrk = rp.tile([128, 1, D], F32, tag="scrk", name="scrk")
                    nc.vector.tensor_mul(scrk, ro_k, ro_k)
                    nc.vector.reduce_sum(sq5[:, HQ:HQ + 1], scrk,
                                         axis=mybir.AxisListType.X)
                    rr5 = rp.tile([128, HQ + 1], F32, tag="rr5", name="rr5")
                    nc.scalar.activation(rr5, sq5, AF.Abs_reciprocal_sqrt,
                                         bias=eps_s[:, :], scale=1.0 / D)

                qn = qnp.tile([128, HQ + 1, D], dt, tag="qn", name="qn")
                for h in range(HQ):
                    nc.vector.tensor_scalar_mul(qn[:, h], ro_q[:, h],
                                                rr5[:, h:h + 1])
                nc.vector.tensor_scalar_mul(qn[:, HQ], ro_k[:, 0],
                                            rr5[:, HQ:HQ + 1])
                if bxp:
                    pend.append((qn[:, 0:HQ].rearrange("p a b -> p (a b)"),
                                 qT[j][:, :, tsub]))
                    pend.append((qn[:, HQ], kT[i]))
                else:
                    for h in range(HQ):
                        pend.append((qn[:, h], qT[(h, j)][:, tsub]))
                    pend.append((qn[:, HQ], kT[i]))

            # ---- stage B + C per tq-slice ----
            def stage_bc(j, nxt):
                nblk = 4 * j + 4
                ynj = yp.tile([128, HQ, 4, 128], ydt, tag="ynj", name="ynj")
                pend_norm = []  # deferred one head for slack

                def drain_norm():
                    for rdr_p, yv_p, h_p in pend_norm:
                        pb = ps_s.tile([128, 512], F32, tag="s", name="pb")
                        nc.tensor.matmul(pb, ones_sq[0:1, :], rdr_p)
                        nc.vector.tensor_mul(
                            ynj[:, h_p].rearrange("p a b -> p (a b)"),
                            yv_p, pb)
                    del pend_norm[:]

                for h in range(HQ):
                    if h < len(nxt):
                        load_strip(nxt[h])
                    pes = []
                    for i in range(nblk):
                        ai = max(0, i - 4 * j) * 128
                        diag = i >= 4 * j
                        psb = ps_s.tile([128, 512], F32, tag="s")
                        qmv = (qT[j][:, h, ai // 128:4] if bxp
                               else qT[(h, j)][:, ai // 128:4])
                        if pemask and diag:
                            nc.tensor.matmul(psb[:, ai:512], kT[i], qmv,
                                             start=True, stop=False)
                            nc.tensor.matmul(psb[:, ai:ai + 128], ident,
                                             mask_s, start=False, stop=True)
                        else:
                            nc.tensor.matmul(psb[:, ai:512], kT[i], qmv)
                        pe = pp.tile([128, 512], dt, tag="pe")
                        nc.scalar.activation(pe[:, ai:512], psb[:, ai:512],
                                             AF.Exp, scale=SCALE)
                        if diag and not pemask:
                            nc.vector.tensor_mul(pe[:, ai:ai + 128],
                                                 pe[:, ai:ai + 128], mask_s)
                        pes.append((pe, ai))
                    pdh = ps_d.tile([1, 512], F32)
                    for i, (pe, ai) in enumerate(pes):
                        nc.tensor.matmul(pdh[:, ai:512], ones_c,
                                         pe[:, ai:512],
                                         start=(i == 0), stop=(i == nblk - 1))
                    rd = bp.tile([1, 512], F32, tag="rd")
                    nc.vector.reciprocal(rd, pdh)
                    rdr = bp.tile([1, 512], dt, tag="rdr")
                    nc.vector.tensor_copy(rdr, rd)
                    if defnorm:
                        drain_norm()
                    pav = ps_a.tile([128, 512], F32)
                    for i, (pe, ai) in enumerate(pes):
                        nc.tensor.matmul(pav[:, ai:512], vS[i],
                                         pe[:, ai:512],
                                         start=(i == 0), stop=(i == nblk - 1))
                    yv = yp.tile([128, 512], dt, tag="yv", name="yv")
                    if h % 2 == 0:
                        nc.scalar.copy(yv, pav)
                    else:
                        nc.vector.tensor_copy(yv, pav)
                    pend_norm.append((rdr, yv, h))
                    if not defnorm:
                        drain_norm()
                    if h < len(nxt):
                        stage_a(nxt[h])
                drain_norm()
                drain_transposes()
                if stages == 'AB':
                    dbg2 = op.tile([128, 512], F32, tag="dbg")
                    nc.vector.tensor_copy(
                        dbg2, ynj[:, 0].rearrange("p a b -> p (a b)"))
                    nc.sync.dma_start(out_d.ap()[ts(j, 128), 0:512], dbg2)
                    return
                for tsub in range(4):
                    otb = None
                    if bigot:
                        otb = op.tile([128, NE, 512], odt, tag="ot",
                                      name="otb")
                    for e in range(NE):
                        pc = ps_s.tile([128, 512], F32, tag="s", name="pc")
                        if fp8c:
                            for hp in range(HQ // 2):
                                nc.tensor.matmul(
                                    pc, ynj[:, 2 * hp:2 * hp + 2, tsub],
                                    wp_s[:, 2 * hp:2 * hp + 2, ts(e, 512)],
                                    start=(hp == 0), stop=(hp == HQ // 2 - 1),
                                    perf_mode=DR)
                        else:
                            for h in range(HQ):
                                nc.tensor.matmul(pc, ynj[:, h, tsub],
                                                 wp_s[:, h, ts(e, 512)],
                                                 start=(h == 0),
                                                 stop=(h == HQ - 1))
                        ot = otb[:, e] if bigot else op.tile(
                            [128, 512], odt, tag="ot", name="ot")
                        if e % 2 == 0:
                            if ot_scale is None:
                                nc.scalar.copy(ot, pc)
                            else:
                                nc.scalar.mul(ot, pc, ot_scale)
                        else:
                            if ot_scale is None:
                                nc.vector.tensor_copy(ot, pc)
                            else:
                                nc.vector.tensor_scalar_mul(ot, pc, ot_scale)
                        if not bigot:
                            deng = (nc.sync if (e % 2 == 0 or not gdma)
                                    else nc.gpsimd)
                            deng.dma_start(
                                out_d.ap()[512 * j + 128 * tsub:
                                           512 * j + 128 * tsub + 128,
                                           ts(e, 512)], ot)
                    if bigot:
                        ste.dma_start(
                            out_d.ap()[512 * j + 128 * tsub:
                                       512 * j + 128 * tsub + 128, :]
                            .rearrange("p (n e) -> p n e", e=512), otb)

            for i in range(min(4, TT)):
                stage_a(i)
            drain_transposes()
            if stages == 'A':
                dbg = op.tile([128, 512], F32, tag="dbg", name="dbg")
                nc.vector.tensor_copy(dbg[:, 0:128], kT[0])
                nc.sync.dma_start(out_d.ap()[0:128, 0:512], dbg)
                continue
            for j in range(TQ):
                if deep:
                    nxt = [i for i in range(4 * j + 4, 4 * j + 8) if i < TT]
                else:
                    nxt = []
                    if j < TQ - 1:
                        for i in range(4 * j + 4, min(4 * j + 8, TT)):
                            load_strip(i)
                stage_bc(j, nxt)
                if not deep and j < TQ - 1:
                    for i in range(4 * j + 4, min(4 * j + 8, TT)):
                        stage_a(i)
                    drain_transposes()
    nc.compile()
    return nc


def make_core_inputs(x, cos, sin, wq, wk, wv, wproj):
    """Full inputs -> list of 8 per-core input dicts (host-side sharding)."""
    bf16 = mybir.dt.np(BF16)
    f8 = mybir.dt.np(FP8)
    xdt = f8 if FP8P else bf16
    pdt = f8 if FP8C else bf16
    wscale = W_SCALE if FP8P else 1.0
    pscale = W_SCALE if FP8C else 1.0
    x = np.asarray(x, dtype=np.float32)
    cos2 = np.ascontiguousarray(np.asarray(cos, np.float32).reshape(-1, D))
    sin2 = np.ascontiguousarray(np.asarray(sin, np.float32).reshape(-1, D))
    wq = np.asarray(wq, np.float32)
    wk = np.asarray(wk, np.float32)
    wv = np.asarray(wv, np.float32)
    wproj = np.asarray(wproj, np.float32)
    B = x.shape[0]
    tri = np.triu(np.ones((128, 128), np.float32))
    if PEMASK:
        mask = np.where(tri > 0, 0.0, -30000.0).astype(np.float32).astype(bf16)
    else:
        mask = tri.astype(bf16)
    ident = np.eye(128, dtype=np.float32).astype(bf16)
    in_maps = []
    xTs = [np.ascontiguousarray(x[b].T).astype(xdt) for b in range(B)]
    for b in range(B):
        for g in range(N_KV):
            wqT = np.ascontiguousarray(
                wq[512 * g:512 * g + 512].T * wscale).astype(xdt)
            wkvT = np.ascontiguousarray(
                np.concatenate([wk[128 * g:128 * g + 128],
                                wv[128 * g:128 * g + 128]],
                               axis=0).T * wscale).astype(xdt)
            wpT = np.ascontiguousarray(
                wproj[:, 512 * g:512 * g + 512].T * pscale).astype(pdt)
            in_maps.append({
                "xT": xTs[b], "wqT": wqT, "wkvT": wkvT, "wpT": wpT,
                "cosd": cos2, "sind": sin2, "maskd": mask, "identd": ident,
            })
    return in_maps


_NC_CACHE = {}


def kernel(x, cos, sin, wq, wk, wv, wproj):
    x = np.asarray(x, dtype=np.float32)
    B, T, C = x.shape
    key = (T, C)
    if key not in _NC_CACHE:
        _NC_CACHE[key] = build_bass(T=T, C=C)
    nc = _NC_CACHE[key]
    in_maps = make_core_inputs(x, cos, sin, wq, wk, wv, wproj)
    res = run_bass_kernel_spmd(nc, in_maps, core_ids=list(range(8)))
    out = np.zeros((B, T, C), dtype=np.float64)
    for b in range(B):
        for g in range(N_KV):
            out[b] += res.results[4 * b + g]["out"].astype(np.float64)
    return out.astype(np.float32)


# revision 27
# speedup vs baseline: 1.3757x; 1.0334x over previous
"""Trainium2 Bass kernel: causal GQA self-attention, RoPE + QK RMS-norm, bf16.

Sharding over 8 NeuronCores: core = 4*b + g (b in {0,1} batch, g in {0..3}
kv-group). Each core computes its 4 q heads + 1 kv head and the partial
c_proj output y_heads @ wproj[:, 512g:512g+512].T of shape [T, C]; the host
sums the 4 partials per batch (the "all-reduce after c_proj" at gather time).

Schedule (deep software pipeline): stage A of strip 4j+4+h is emitted inside
stage B(j) between heads h and h+1, so the serial per-strip chain
(PE proj -> DVE rope+rsqrt -> xbar transpose) hides under the PE-heavy
attention sweeps and every engine queue keeps flowing. Choices that matter:
 - rsqrt for the QK RMS-norm runs ENTIRELY on DVE (bit-trick seed + 2
   Newton steps on [128,5] tiles): the ACT engine then only ever runs
   Exp/Copy, which share one activation-table set -> zero ACT_TABLE_LOADs
   after the first.
 - causal mask is an additive -30000 matmul folded into the scores psum
   accumulation group on PE (no post-exp masking work on DVE/ACT).
 - q transposes are batched: one 3D-dst xbar DMA per strip covers all 4
   heads ([t,(h d)] -> [d,h,t]).
 - output stores are 4 batched [128,2048] DMAs per slice on the gpsimd
   (SWDGE) queue; cos/sin loads ride the same queue ahead of them; the SP
   (HWDGE) queue carries only x strips + transposes. In-order DMA queues
   head-of-line block on data dependencies, so queue assignment is part of
   the schedule.
 - fp8 (DoubleRow) paths exist but are OFF: every naive e4m3 insertion
   measures ~2.8e-2 rel err alone (gate 2e-2); error-feedback splits cost
   back the 2x.
"""

import math
from contextlib import ExitStack

import numpy as np

import concourse.bass as bass
import concourse.mybir as mybir
import concourse.tile as tile
from concourse import bacc
from concourse.bass import ts
from concourse.bass_utils import run_bass_kernel_spmd

F32 = mybir.dt.float32
I32 = mybir.dt.int32
BF16 = mybir.dt.bfloat16
FP8 = mybir.dt.float8e4
N_HEAD = 16
N_KV = 4
D = 128
RMS_EPS = float(np.finfo(np.float32).eps)
SCALE = 1.0 / math.sqrt(D)

FP8P = False         # x/wq/wkv in fp8, DoubleRow projections (see docstring)
FP8C = False         # wproj/ynj in fp8, DoubleRow c_proj
W_SCALE = 64.0
OUT_DT = "bf16"      # halves output-store DMA; measured rel err impact
                     # 3.6e-3 -> 4.5e-3 in the numpy quantization model
PEMASK = True        # mask = additive -30000 folded into scores matmul group

ALU = mybir.AluOpType
AF = mybir.ActivationFunctionType


def build_bass(T=2048, C=2048, HQ=4, E=2048, rep=1,
               pscfg=None, pbufs=16, obufs=3,
               stages='ABC', wsplit=4, gdma=True,
               defnorm=True, fp8p=None, fp8c=None, outdt=None,
               nrsqrt=True, deep=True, bigot=True, bxp=True,
               pemask=True, csq='gpsimd', stq='gpsimd', nriter=2):
    TT, CT, NE, TQ = T // 128, C // 128, E // 512, T // 512
    HD = HQ * 128
    if fp8p is None:
        fp8p = FP8P
    if fp8c is None:
        fp8c = FP8C
    if outdt is None:
        outdt = OUT_DT
    dt = BF16
    xdt = FP8 if fp8p else BF16
    ydt = FP8 if fp8c else BF16
    odt = F32 if outdt == "f32" else BF16
    os_val = (1.0 / W_SCALE) if fp8p else 1.0
    ot_scale = (1.0 / W_SCALE) if fp8c else None
    if pscfg is None:
        pscfg = (2, 2, 1, 2, 1)  # ps_s, pq, pkv, ps_a, ps_d (8 banks)
    sb_, qb_, kb_, ab_, db_ = pscfg

    nc = bacc.Bacc("TRN2", target_bir_lowering=False)
    xT_d = nc.dram_tensor("xT", [C, T], xdt, kind="ExternalInput")
    wqT_d = nc.dram_tensor("wqT", [C, HD], xdt, kind="ExternalInput")
    wkvT_d = nc.dram_tensor("wkvT", [C, 256], xdt, kind="ExternalInput")
    wpT_d = nc.dram_tensor("wpT", [HD, E], ydt, kind="ExternalInput")
    cos_d = nc.dram_tensor("cosd", [T, D], F32, kind="ExternalInput")
    sin_d = nc.dram_tensor("sind", [T, D], F32, kind="ExternalInput")
    mask_d = nc.dram_tensor("maskd", [128, 128], dt, kind="ExternalInput")
    id_d = nc.dram_tensor("identd", [128, 128], dt, kind="ExternalInput")
    out_d = nc.dram_tensor("out", [T, E], odt, kind="ExternalOutput")

    DR = mybir.MatmulPerfMode.DoubleRow
    assert pemask == PEMASK, "host mask content must match pemask"

    with tile.TileContext(nc) as tc, ExitStack() as ctx:
        P = lambda **kw: ctx.enter_context(tc.tile_pool(**kw))
        wp = P(name="w", bufs=1)            # persistent weights/constants
        xp = P(name="x", bufs=8)            # xT strips
        csp = P(name="cs", bufs=8)          # cos/sin tiles
        rp = P(name="rope", bufs=3)         # rope scratch
        qnp = P(name="qn", bufs=3)          # normalized q/k (pre-transpose)
        pp = P(name="p", bufs=pbufs)        # exp(P) tiles (all blocks alive)
        bp = P(name="bc", bufs=2)           # denominators / bcast
        yp = P(name="y", bufs=3)            # per-head unnormalized y^T
        op = P(name="o", bufs=obufs)        # output staging
        ps_s = P(name="ps_s", bufs=sb_, space="PSUM")   # scores/pb/cproj
        ps_q = P(name="ps_q", bufs=qb_, space="PSUM")   # q projection
        ps_k = P(name="ps_k", bufs=kb_, space="PSUM")   # kv projection
        ps_a = P(name="ps_a", bufs=ab_, space="PSUM")   # AV accumulators
        ps_d = P(name="ps_d", bufs=db_, space="PSUM")   # denominators

        engq = {"gpsimd": nc.gpsimd, "sp": nc.sync,
                "act": nc.scalar, "dve": nc.vector}
        cse = engq[csq] if gdma else nc.sync
        ste = engq[stq] if gdma else nc.sync

        xT_r = xT_d.ap().rearrange("(n p) t -> p n t", p=128)
        xs_t, cs_t = {}, {}

        def load_strip(i):
            xs = xp.tile([128, CT, 128], xdt, name="xs", tag="xs")
            nc.sync.dma_start(xs, xT_r[:, :, ts(i, 128)])
            cst = csp.tile([128, D], F32, tag="cos", name="cst")
            cse.dma_start(cst, cos_d.ap()[ts(i, 128), :])
            snt = csp.tile([128, D], F32, tag="sin", name="snt")
            cse.dma_start(snt, sin_d.ap()[ts(i, 128), :])
            xs_t[i] = xs
            cs_t[i] = (cst, snt)

        # ---- startup DMAs: first strip, then constants + weights ----
        load_strip(0)
        mask_s = wp.tile([128, 128], dt)
        nc.sync.dma_start(mask_s, mask_d.ap())
        ident = wp.tile([128, 128], dt)
        nc.sync.dma_start(ident, id_d.ap())
        wq_s = wp.tile([128, CT, HD], xdt)
        wkv_s = wp.tile([128, CT, 256], xdt)
        wq_r = wqT_d.ap().rearrange("(n p) m -> p n m", p=128)
        wkv_r = wkvT_d.ap().rearrange("(n p) m -> p n m", p=128)
        csz = CT // wsplit
        for w in range(wsplit):
            cs0 = w * csz
            nc.sync.dma_start(wq_s[:, cs0:cs0 + csz], wq_r[:, cs0:cs0 + csz])
            nc.sync.dma_start(wkv_s[:, cs0:cs0 + csz], wkv_r[:, cs0:cs0 + csz])
        for i in range(1, min(4, TT)):
            load_strip(i)
        wp_s = wp.tile([128, HQ, E], ydt)
        wp_r = wpT_d.ap().rearrange("(n p) m -> p n m", p=128)
        wp_loaded = [False]
        ones_c = wp.tile([128, 1], dt, name="ones_c", tag="ones_c")
        nc.vector.memset(ones_c, 1.0)
        ones_sq = wp.tile([128, 128], dt, name="ones_sq", tag="ones_sq")
        nc.vector.memset(ones_sq, os_val)
        eps_s = wp.tile([128, 1], F32)
        nc.vector.memset(eps_s, RMS_EPS)

        def bcast(ap, axis, n):
            a = list(ap.ap)
            a.insert(axis, [0, n])
            return bass.AP(tensor=ap.tensor, offset=ap.offset, ap=a)

        if True:  # tiles/closures shared by all reps (flat group stream)
            if bxp:
                qT = {j: wp.tile([128, HQ, 4, 128], dt, tag=f"qTj{j}",
                                 name=f"qTj{j}") for j in range(TQ)}
            else:
                qT = {}
                for h in range(HQ):
                    for j in range(TQ):
                        qT[(h, j)] = wp.tile([128, 4, 128], dt,
                                             tag=f"qT{h}_{j}",
                                             name=f"qT{h}_{j}")
            kT = [wp.tile([128, 128], dt, tag=f"kT{i}", name=f"kT{i}")
                  for i in range(TT)]
            vS = [wp.tile([128, 128], dt, tag=f"vS{i}", name=f"vS{i}")
                  for i in range(TT)]

            pend = []  # deferred transposes: (src_ap, dst)

            def drain_transposes():
                for src_ap, dst in pend:
                    nc.sync.dma_start(dst, src_ap, transpose=True)
                del pend[:]

            def rope(src, nh, cst, snt, qn, qo):
                """src: psum [128, nh, 128] -> rope'd copy in qn[:, qo:qo+nh]
                (f32 scratch), returns sumsq written later; here returns ro."""
                ro = rp.tile([128, nh, D], F32, tag=f"ro{qo}", name="ro")
                nc.vector.tensor_mul(ro, src, bcast(cst[:, :], 1, nh))
                tmp = rp.tile([128, nh, 64], F32, tag=f"tm{qo}", name="tmp")
                nc.vector.tensor_mul(tmp, src[:, :, 64:128],
                                     bcast(snt[:, 0:64], 1, nh))
                nc.vector.tensor_sub(ro[:, :, 0:64], ro[:, :, 0:64], tmp)
                tmp2 = rp.tile([128, nh, 64], F32, tag=f"t2{qo}", name="tmp2")
                nc.vector.tensor_mul(tmp2, src[:, :, 0:64],
                                     bcast(snt[:, 64:128], 1, nh))
                nc.vector.tensor_add(ro[:, :, 64:128], ro[:, :, 64:128],
                                     tmp2)
                return ro

            MAGIC = 0x5F3759DF

            def nr_rsqrt(rr, v):
                """rr = 1/sqrt(v) elementwise on DVE only ([128, n] tiles).

                Bit-trick seed y0 via (M2 + ~i) >> 1 (= magic - (i>>1) up to
                1 ulp of the seed), then `nriter` Newton steps; no ACT
                involvement so the activation table stays on the Exp set.
                """
                n = v.shape[1]
                y = rp.tile([128, n], F32, tag="nr_y", name="nr_y")
                vb = v.bitcast(I32)
                yb = y.bitcast(I32)
                # walrus requires op0/op1 of one tensor_scalar to share a
                # class (bitwise vs arith), and there is no reversed
                # subtract: use magic - (i>>1) = ~(i>>1) + (magic+1)
                nc.vector.tensor_scalar(yb, vb, 1, 0xFFFFFFFF,
                                        ALU.logical_shift_right,
                                        ALU.bitwise_xor)
                nc.vector.tensor_scalar(yb, yb, MAGIC + 1, None, ALU.add)
                t = rp.tile([128, n], F32, tag="nr_t", name="nr_t")
                for _ in range(nriter):
                    nc.vector.tensor_mul(t, y, y)
                    nc.vector.tensor_mul(t, t, v)
                    nc.vector.tensor_scalar(t, t, -0.5, 1.5,
                                            ALU.mult, ALU.add)
                    nc.vector.tensor_mul(y, y, t)
                nc.vector.tensor_copy(rr, y)

            def stage_a(i):
                drain_transposes()
                xs = xs_t[i]
                cst, snt = cs_t[i]
                if not wp_loaded[0] and i >= 1:
                    # all HQ wproj chunks must be queued on the SP FIFO
                    # before bc(0)'s output stores (stage C(0) reads them)
                    hi = HQ if i >= 3 else i
                    for w in range(i - 1, hi):
                        nc.sync.dma_start(wp_s[:, w], wp_r[:, w])
                    if i >= 3:
                        wp_loaded[0] = True

                pq = ps_q.tile([128, HD], F32, tag="pq", name="pq")
                pkv = ps_k.tile([128, 256], F32, tag="pkv", name="pkv")
                if fp8p:
                    n2 = CT // 2
                    for c in range(n2):
                        sl = slice(2 * c, 2 * c + 2)
                        nc.tensor.matmul(pq, xs[:, sl], wq_s[:, sl],
                                         start=(c == 0), stop=(c == n2 - 1),
                                         perf_mode=DR)
                        nc.tensor.matmul(pkv, xs[:, sl], wkv_s[:, sl],
                                         start=(c == 0), stop=(c == n2 - 1),
                                         perf_mode=DR)
                else:
                    for c in range(CT):
                        nc.tensor.matmul(pq, xs[:, c], wq_s[:, c],
                                         start=(c == 0), stop=(c == CT - 1))
                        nc.tensor.matmul(pkv, xs[:, c], wkv_s[:, c],
                                         start=(c == 0), stop=(c == CT - 1))
                nc.scalar.copy(vS[i], pkv[:, 128:256])
                j, tsub = i // 4, i % 4

                ro_q = rope(pq[:].rearrange("p (h d) -> p h d", d=D), HQ,
                            cst, snt, None, 0)
                ro_k = rope(pkv[:, 0:128].rearrange("p (h d) -> p h d", d=D),
                            1, cst, snt, None, 8)
                sq5 = rp.tile([128, HQ + 1], F32, tag="sq5", name="sq5")
                if nrsqrt:
                    # v = mean(ro^2) + eps, rsqrt on DVE
                    scr = rp.tile([128, HQ, D], F32, tag="scr", name="scr")
                    nc.vector.tensor_mul(scr, ro_q, ro_q)
                    nc.vector.reduce_sum(sq5[:, 0:HQ], scr,
                                         axis=mybir.AxisListType.X)
                    scrk = rp.tile([128, 1, D], F32, tag="scrk", name="scrk")
                    nc.vector.tensor_mul(scrk, ro_k, ro_k)
                    nc.vector.reduce_sum(sq5[:, HQ:HQ + 1], scrk,
                                         axis=mybir.AxisListType.X)
                    nc.vector.tensor_scalar(sq5, sq5, 1.0 / D, RMS_EPS,
                                            ALU.mult, ALU.add)
                    rr5 = rp.tile([128, HQ + 1], F32, tag="rr5", name="rr5")
                    nr_rsqrt(rr5, sq5)
                else:
                    scr = rp.tile([128, HQ, D], F32, tag="scr", name="scr")
                    nc.vector.tensor_mul(scr, ro_q, ro_q)
                    nc.vector.reduce_sum(sq5[:, 0:HQ], scr,
                                         axis=mybir.AxisListType.X)
                    scrk = rp.tile([128, 1, D], F32, tag="scrk", name="scrk")
                    nc.vector.tensor_mul(scrk, ro_k, ro_k)
                    nc.vector.reduce_sum(sq5[:, HQ:HQ + 1], scrk,
                                         axis=mybir.AxisListType.X)
                    rr5 = rp.tile([128, HQ + 1], F32, tag="rr5", name="rr5")
                    nc.scalar.activation(rr5, sq5, AF.Abs_reciprocal_sqrt,
                                         bias=eps_s[:, :], scale=1.0 / D)

                qn = qnp.tile([128, HQ + 1, D], dt, tag="qn", name="qn")
                for h in range(HQ):
                    nc.vector.tensor_scalar_mul(qn[:, h], ro_q[:, h],
                                                rr5[:, h:h + 1])
                nc.vector.tensor_scalar_mul(qn[:, HQ], ro_k[:, 0],
                                            rr5[:, HQ:HQ + 1])
                if bxp:
                    pend.append((qn[:, 0:HQ].rearrange("p a b -> p (a b)"),
                                 qT[j][:, :, tsub]))
                    pend.append((qn[:, HQ], kT[i]))
                else:
                    for h in range(HQ):
                        pend.append((qn[:, h], qT[(h, j)][:, tsub]))
                    pend.append((qn[:, HQ], kT[i]))

            # ---- stage B + C per tq-slice ----
            def stage_bc(j, nxt):
                nblk = 4 * j + 4
                ynj = yp.tile([128, HQ, 4, 128], ydt, tag="ynj", name="ynj")
                pend_norm = []  # deferred one head for slack

                def drain_norm():
                    for rdr_p, yv_p, h_p in pend_norm:
                        pb = ps_s.tile([128, 512], F32, tag="s", name="pb")
                        nc.tensor.matmul(pb, ones_sq[0:1, :], rdr_p)
                        nc.vector.tensor_mul(
                            ynj[:, h_p].rearrange("p a b -> p (a b)"),
                            yv_p, pb)
                    del pend_norm[:]

                for h in range(HQ):
                    if h < len(nxt):
                        load_strip(nxt[h])
                    pes = []
                    for i in range(nblk):
                        ai = max(0, i - 4 * j) * 128
                        diag = i >= 4 * j
                        psb = ps_s.tile([128, 512], F32, tag="s")
                        qmv = (qT[j][:, h, ai // 128:4] if bxp
                               else qT[(h, j)][:, ai // 128:4])
                        if pemask and diag:
                            nc.tensor.matmul(psb[:, ai:512], kT[i], qmv,
                                             start=True, stop=False)
                            nc.tensor.matmul(psb[:, ai:ai + 128], ident,
                                             mask_s, start=False, stop=True)
                        else:
                            nc.tensor.matmul(psb[:, ai:512], kT[i], qmv)
                        pe = pp.tile([128, 512], dt, tag="pe")
                        nc.scalar.activation(pe[:, ai:512], psb[:, ai:512],
                                             AF.Exp, scale=SCALE)
                        if diag and not pemask:
                            nc.vector.tensor_mul(pe[:, ai:ai + 128],
                                                 pe[:, ai:ai + 128], mask_s)
                        pes.append((pe, ai))
                    pdh = ps_d.tile([1, 512], F32)
                    for i, (pe, ai) in enumerate(pes):
                        nc.tensor.matmul(pdh[:, ai:512], ones_c,
                                         pe[:, ai:512],
                                         start=(i == 0), stop=(i == nblk - 1))
                    rd = bp.tile([1, 512], F32, tag="rd")
                    nc.vector.reciprocal(rd, pdh)
                    rdr = bp.tile([1, 512], dt, tag="rdr")
                    nc.vector.tensor_copy(rdr, rd)
                    if defnorm:
                        drain_norm()
                    pav = ps_a.tile([128, 512], F32)
                    for i, (pe, ai) in enumerate(pes):
                        nc.tensor.matmul(pav[:, ai:512], vS[i],
                                         pe[:, ai:512],
                                         start=(i == 0), stop=(i == nblk - 1))
                    yv = yp.tile([128, 512], dt, tag="yv", name="yv")
                    if h % 2 == 0:
                        nc.scalar.copy(yv, pav)
                    else:
                        nc.vector.tensor_copy(yv, pav)
                    pend_norm.append((rdr, yv, h))
                    if not defnorm:
                        drain_norm()
                    if h < len(nxt):
                        stage_a(nxt[h])
                drain_norm()
                drain_transposes()
                if stages == 'AB':
                    dbg2 = op.tile([128, 512], F32, tag="dbg")
                    nc.vector.tensor_copy(
                        dbg2, ynj[:, 0].rearrange("p a b -> p (a b)"))
                    nc.sync.dma_start(out_d.ap()[ts(j, 128), 0:512], dbg2)
                    return
                for tsub in range(4):
                    otb = None
                    if bigot:
                        otb = op.tile([128, NE, 512], odt, tag="ot",
                                      name="otb")
                    for e in range(NE):
                        pc = ps_s.tile([128, 512], F32, tag="s", name="pc")
                        if fp8c:
                            for hp in range(HQ // 2):
                                nc.tensor.matmul(
                                    pc, ynj[:, 2 * hp:2 * hp + 2, tsub],
                                    wp_s[:, 2 * hp:2 * hp + 2, ts(e, 512)],
                                    start=(hp == 0), stop=(hp == HQ // 2 - 1),
                                    perf_mode=DR)
                        else:
                            for h in range(HQ):
                                nc.tensor.matmul(pc, ynj[:, h, tsub],
                                                 wp_s[:, h, ts(e, 512)],
                                                 start=(h == 0),
                                                 stop=(h == HQ - 1))
                        ot = otb[:, e] if bigot else op.tile(
                            [128, 512], odt, tag="ot", name="ot")
                        if e % 2 == 0:
                            if ot_scale is None:
                                nc.scalar.copy(ot, pc)
                            else:
                                nc.scalar.mul(ot, pc, ot_scale)
                        else:
                            if ot_scale is None:
                                nc.vector.tensor_copy(ot, pc)
                            else:
                                nc.vector.tensor_scalar_mul(ot, pc, ot_scale)
                        if not bigot:
                            deng = (nc.sync if (e % 2 == 0 or not gdma)
                                    else nc.gpsimd)
                            deng.dma_start(
                                out_d.ap()[512 * j + 128 * tsub:
                                           512 * j + 128 * tsub + 128,
                                           ts(e, 512)], ot)
                    if bigot:
                        ste.dma_start(
                            out_d.ap()[512 * j + 128 * tsub:
                                       512 * j + 128 * tsub + 128, :]
                            .rearrange("p (n e) -> p n e", e=512), otb)

            for i in range(min(4, TT)):
                stage_a(i)
            drain_transposes()
            if stages == 'A':
                dbg = op.tile([128, 512], F32, tag="dbg", name="dbg")
                nc.vector.tensor_copy(dbg[:, 0:128], kT[0])
                nc.sync.dma_start(out_d.ap()[0:128, 0:512], dbg)
            else:
                # flat stream of rep*TQ groups: group g+1's strips are
                # prefetched/computed inside bc(g), ACROSS rep boundaries,
                # so the rep-slope has no per-rep pipeline refill
                NG = rep * TQ
                for g in range(NG):
                    j = g % TQ
                    if g + 1 < NG:
                        nxts = [(4 * (g + 1) + k) % TT for k in range(4)]
                    else:
                        nxts = []
                    if deep:
                        stage_bc(j, nxts)
                    else:
                        for i in nxts:
                            load_strip(i)
                        stage_bc(j, [])
                        for i in nxts:
                            stage_a(i)
                        drain_transposes()
    nc.compile()
    return nc


def make_core_inputs(x, cos, sin, wq, wk, wv, wproj):
    """Full inputs -> list of 8 per-core input dicts (host-side sharding)."""
    bf16 = mybir.dt.np(BF16)
    f8 = mybir.dt.np(FP8)
    xdt = f8 if FP8P else bf16
    pdt = f8 if FP8C else bf16
    wscale = W_SCALE if FP8P else 1.0
    pscale = W_SCALE if FP8C else 1.0
    x = np.asarray(x, dtype=np.float32)
    cos2 = np.ascontiguousarray(np.asarray(cos, np.float32).reshape(-1, D))
    sin2 = np.ascontiguousarray(np.asarray(sin, np.float32).reshape(-1, D))
    wq = np.asarray(wq, np.float32)
    wk = np.asarray(wk, np.float32)
    wv = np.asarray(wv, np.float32)
    wproj = np.asarray(wproj, np.float32)
    B = x.shape[0]
    tri = np.triu(np.ones((128, 128), np.float32))
    if PEMASK:
        mask = np.where(tri > 0, 0.0, -30000.0).astype(np.float32).astype(bf16)
    else:
        mask = tri.astype(bf16)
    ident = np.eye(128, dtype=np.float32).astype(bf16)
    in_maps = []
    xTs = [np.ascontiguousarray(x[b].T).astype(xdt) for b in range(B)]
    for b in range(B):
        for g in range(N_KV):
            wqT = np.ascontiguousarray(
                wq[512 * g:512 * g + 512].T * wscale).astype(xdt)
            wkvT = np.ascontiguousarray(
                np.concatenate([wk[128 * g:128 * g + 128],
                                wv[128 * g:128 * g + 128]],
                               axis=0).T * wscale).astype(xdt)
            wpT = np.ascontiguousarray(
                wproj[:, 512 * g:512 * g + 512].T * pscale).astype(pdt)
            in_maps.append({
                "xT": xTs[b], "wqT": wqT, "wkvT": wkvT, "wpT": wpT,
                "cosd": cos2, "sind": sin2, "maskd": mask, "identd": ident,
            })
    return in_maps


_NC_CACHE = {}


def kernel(x, cos, sin, wq, wk, wv, wproj):
    x = np.asarray(x, dtype=np.float32)
    B, T, C = x.shape
    key = (T, C)
    if key not in _NC_CACHE:
        _NC_CACHE[key] = build_bass(T=T, C=C)
    nc = _NC_CACHE[key]
    in_maps = make_core_inputs(x, cos, sin, wq, wk, wv, wproj)
    res = run_bass_kernel_spmd(nc, in_maps, core_ids=list(range(8)))
    out = np.zeros((B, T, C), dtype=np.float64)
    for b in range(B):
        for g in range(N_KV):
            out[b] += res.results[4 * b + g]["out"].astype(np.float64)
    return out.astype(np.float32)
